# revision 1
# baseline (speedup 1.0000x reference)
"""LocationAttention Trainium2 kernel (nn_LocationAttention_83485574300223).

out[b,t,:] = sum_{s<=t} a[b,s] x[b,s,:] / (sum_{s<=t} a[b,s] + eps),
a = exp(x @ w + b).

Data-parallel over batch: 16 -> 2 per core, 8 cores. Causal prefix sums as
triangular matmuls on the TensorEngine in float32r (TF32-class, full rate at
N=512): 128-token chunks, superblocks of 4 chunks. Inter-superblock num carry
via column-sum matmuls accumulated in a dedicated PSUM bank (PE-only chain);
den via per-superblock column-sum matmul + scan chain. Single streaming
pipeline, no batch-level barriers.
"""
import numpy as np

import concourse.bass as bass
import concourse.tile as tile
from concourse import mybir
from concourse.bass_utils import run_bass_kernel_spmd

B, S, H = 16, 4096, 512
NCORES = 8
BPC = B // NCORES  # batch elements per core
P = 128            # partitions == chunk length
CPB = S // P       # chunks per batch element (32)
GF = 4             # chunks per superblock
NG = CPB // GF     # superblocks per batch element (8)

F32 = mybir.dt.float32
F32R = mybir.dt.float32r
BF16 = mybir.dt.bfloat16
AF = mybir.ActivationFunctionType
ALU = mybir.AluOpType
AX = mybir.AxisListType


def _split_multiwaits(nc, limit=1):
    """This walrus build accepts at most one sync-wait per instruction.
    Split extras into preceding single-wait NoOps on the same engine."""
    for fn in nc.m.functions:
        for bb in fn.blocks:
            out = []
            changed = False
            for ins in bb.instructions:
                si = getattr(ins, "sync_info", None)
                waits = list(si.on_wait) if (si is not None and si.on_wait) else []
                if len(waits) > limit:
                    extra, keep = waits[:-limit], waits[-limit:]
                    for i, w in enumerate(extra):
                        nop = mybir.InstNoOp(name=f"{ins.name}-ws{i}", ins=[], outs=[])
                        nop.engine = ins.engine
                        nop.sync_info = mybir.SyncInfo(on_wait=[w], on_update=[])
                        out.append(nop)
                    si.on_wait = keep
                    changed = True
                out.append(ins)
            if changed:
                try:
                    bb.instructions = out
                except Exception:
                    bb.instructions.clear()
                    bb.instructions.extend(out)


def _build():
    nc = bass.Bass()
    x = nc.declare_dram_parameter("x", [BPC, S, H], F32, isOutput=False)
    wb4 = nc.declare_dram_parameter("wb4", [P, GF, H], F32, isOutput=False)
    tri = nc.declare_dram_parameter("tri", [P, P], F32, isOutput=False)
    bsc = nc.declare_dram_parameter("bsc", [1, 1], F32, isOutput=False)
    out = nc.declare_dram_parameter("out", [BPC, S, H], F32, isOutput=True)

    with tile.TileContext(nc) as tc:
        with (
            tc.tile_pool(name="singles", bufs=1) as singles,
            tc.tile_pool(name="xp", bufs=6) as xp,
            tc.tile_pool(name="xwp", bufs=3) as xwp,
            tc.tile_pool(name="outp", bufs=3) as outp,
            tc.tile_pool(name="lhsp", bufs=8) as lhsp,
            tc.tile_pool(name="smallp", bufs=3) as smallp,
            tc.tile_pool(name="carryp", bufs=2) as carryp,
            tc.tile_pool(name="nps", bufs=5, space="PSUM") as nps,
            tc.tile_pool(name="dps", bufs=1, space="PSUM") as dps,
            tc.tile_pool(name="pcp", bufs=2, space="PSUM") as pcp,
        ):
            # ---- constants ----
            wb4_t = singles.tile([P, GF, H], BF16)
            nc.gpsimd.dma_start(out=wb4_t, in_=wb4[:])
            tri_t = singles.tile([P, P], F32)
            nc.sync.dma_start(out=tri_t, in_=tri[:])
            tri_b = singles.tile([P, P], BF16)
            nc.gpsimd.dma_start(out=tri_b, in_=tri[:])
            b_sb = singles.tile([P, 1], F32)
            nc.gpsimd.dma_start(out=b_sb, in_=bsc[:].to_broadcast([P, 1]))
            ones128 = singles.tile([P, P], F32)
            nc.vector.memset(ones128[:], 1.0)
            ones128_b = singles.tile([P, P], BF16)
            nc.vector.memset(ones128_b[:], 1.0)
            ones_row_f = singles.tile([1, P], F32)
            nc.vector.memset(ones_row_f[:], 1.0)
            ones_row_b = singles.tile([1, P], BF16)
            nc.vector.memset(ones_row_b[:], 1.0)
            zeros_t = singles.tile([1, CPB], F32)
            nc.vector.memset(zeros_t[:], 0.0)

            xgs = [x[bi].rearrange("(g f p) h -> g p f h", p=P, f=GF) for bi in range(BPC)]
            ogs = [out[bi].rearrange("(g f p) h -> g p f h", p=P, f=GF) for bi in range(BPC)]

            carry = None       # [1, H] f32r, running num prefix total
            prev_dexcl = None  # [1, GF+1] f32, col GF = running den total
            pending = None     # (og_ap, psums, r4) of previous superblock
            NT = BPC * NG
            xts = {}

            def _load(tt):
                tbi, tk = divmod(tt, NG)
                xtile = xp.tile([P, GF, H], BF16, tag="xt", name=f"xt_{tt}")
                nc.gpsimd.dma_start(out=xtile, in_=xgs[tbi][tk])
                xts[tt] = xtile

            _load(0)
            _load(1)
            for t in range(NT):
                bi, k = divmod(t, NG)
                xg, og = xgs[bi], ogs[bi]
                if k == 0:
                    carry = None
                    prev_dexcl = None
                if True:
                    # -- load + x@w partial products --
                    if t + 2 < NT:
                        _load(t + 2)
                    xt = xts.pop(t)
                    xw = xwp.tile([P, GF, H], BF16, tag="xw", name=f"xw_{bi}_{k}")
                    p4 = smallp.tile([P, GF], F32, tag="p4", name=f"p4_{bi}_{k}")
                    for f in range(GF):  # fused mul+rowsum on DVE
                        nc.vector.scalar_tensor_tensor(
                            out=xw[:, f, :],
                            in0=xt[:, f, :],
                            scalar=1.0,
                            in1=wb4_t[:, f, :],
                            op0=ALU.mult,
                            op1=ALU.mult,
                            accum_out=p4[:, f : f + 1],
                        )
                    a4 = smallp.tile([P, GF], F32, tag="a4", name=f"a4_{bi}_{k}")
                    nc.scalar.activation(
                        out=a4[:], in_=p4[:], func=AF.Exp, bias=b_sb[:, 0:1]
                    )

                    if pending is not None:
                        pog, ppsums, pr4 = pending
                        pog_t = outp.tile([P, GF, H], F32, tag="og", name=f"og_{t}")
                        for j in range(GF):
                            nc.scalar.activation(
                                out=pog_t[:, j, :], in_=ppsums[j][:], func=AF.Copy,
                                scale=pr4[:, j : j + 1],
                            )
                        nc.sync.dma_start(out=pog, in_=pog_t)
                        pending = None

                    # -- num lhsT builds (DVE) --
                    trias = []
                    for j in range(GF):
                        tria = lhsp.tile([P, P], BF16, tag="tria", name=f"tria_{bi}_{k}_{j}")
                        nc.vector.tensor_scalar_mul(
                            tria[:], tri_b[:], a4[:, j : j + 1]
                        )
                        trias.append(tria)
                    abcs = []
                    for i in range(GF - 1):
                        abc = lhsp.tile([P, P], BF16, tag="abc", name=f"abc_{bi}_{k}_{i}")
                        nc.vector.tensor_scalar_mul(
                            abc[:], ones128_b[:], a4[:, i : i + 1]
                        )
                        abcs.append(abc)

                    # -- den for this superblock --
                    den_ps = dps.tile([P, 2 * GF], F32, tag="den", name=f"dps_{bi}_{k}")
                    nc.tensor.matmul(  # chunk totals (col-sums of a4)
                        den_ps[0:1, GF : 2 * GF], ones128[:, 0:1], a4[:],
                        start=True, stop=True,
                    )
                    nc.tensor.matmul(  # chunk-local cumsums
                        den_ps[:, 0:GF], tri_t[:], a4[:],
                        start=True, stop=False, skip_group_check=True,
                    )
                    dexcl = smallp.tile([1, GF + 1], F32, tag="dexcl", name=f"dex_{bi}_{k}")
                    if prev_dexcl is None:
                        nc.vector.memset(dexcl[0:1, 0:1], 0.0)
                    else:
                        nc.vector.tensor_copy(
                            dexcl[0:1, 0:1], prev_dexcl[0:1, GF : GF + 1]
                        )
                    nc.vector.tensor_tensor_scan(
                        out=dexcl[0:1, 1 : GF + 1],
                        data0=den_ps[0:1, GF : 2 * GF],
                        data1=zeros_t[0:1, 0:GF],
                        initial=dexcl[0:1, 0:1],
                        op0=ALU.add,
                        op1=ALU.add,
                    )
                    prev_dexcl = dexcl
                    nc.tensor.matmul(
                        den_ps[:, 0:GF], ones_row_f[:], dexcl[0:1, 0:GF],
                        start=False, stop=True,
                    )
                    # r = 1/den straight from PSUM (den >> eps, eps dropped)
                    r4 = smallp.tile([P, GF], F32, tag="r4", name=f"r4_{bi}_{k}")
                    nc.vector.reciprocal(r4[:], den_ps[:, 0:GF])

                    # -- num matmuls --
                    psums = [
                        nps.tile([P, H], F32, tag="ps", name=f"ps_{bi}_{k}_{j}")
                        for j in range(GF)
                    ]
                    for j in range(GF):
                        mms = [(trias[j][:], xt[:, j, :])]
                        for i in range(j):
                            mms.append((abcs[i][:], xt[:, i, :]))
                        if carry is not None:
                            mms.append((ones_row_b[:], carry[:]))
                        n = len(mms)
                        for m, (lhsT, rhs) in enumerate(mms):
                            nc.tensor.matmul(
                                psums[j][:], lhsT, rhs,
                                start=(m == 0), stop=(m == n - 1),
                            )

                    # -- carry: per-batch PSUM-resident accumulator --
                    # col-sum lhsT = column 127 of tria_i (tri[:,127]==1 -> a_i)
                    if k < NG - 1:
                        if k == 0:
                            pc = pcp.tile([1, H], F32, tag="pc", name=f"pc_{bi}")
                        for i in range(GF):
                            nc.tensor.matmul(
                                pc[:], trias[i][:, 127:128], xt[:, i, :],
                                start=(k == 0 and i == 0),
                                stop=(k == NG - 2 and i == GF - 1),
                                skip_group_check=True,
                            )
                        new_carry = carryp.tile([1, H], BF16, tag="carry", name=f"carry_{bi}_{k}")
                        nc.vector.tensor_copy(new_carry[:], pc[:])
                        carry = new_carry
                    else:
                        carry = None

                    # -- scale + store: deferred to next iteration (sw pipeline) --
                    pending = (og[k], psums, r4)

            if pending is not None:
                pog, ppsums, pr4 = pending
                pog_t = outp.tile([P, GF, H], F32, tag="og", name="og_final")
                for j in range(GF):
                    nc.scalar.activation(
                        out=pog_t[:, j, :], in_=ppsums[j][:], func=AF.Copy,
                        scale=pr4[:, j : j + 1],
                    )
                nc.sync.dma_start(out=pog, in_=pog_t)

    _split_multiwaits(nc)
    return nc


_NC = None


def _get_nc():
    global _NC
    if _NC is None:
        _NC = _build()
    return _NC


def _prep_in_maps(input_data, w, b):
    x = np.ascontiguousarray(np.asarray(input_data, dtype=np.float32))
    assert x.shape == (B, S, H), x.shape
    w = np.asarray(w, dtype=np.float32).reshape(H)
    b = np.float32(np.asarray(b, dtype=np.float32).reshape(()))
    wb4 = np.ascontiguousarray(np.broadcast_to(w, (P, GF, H))).astype(np.float32)
    tri = np.triu(np.ones((P, P), dtype=np.float32))
    bsc = np.full((1, 1), b, dtype=np.float32)
    return [
        {
            "x": np.ascontiguousarray(x[i * BPC : (i + 1) * BPC]),
            "wb4": wb4,
            "tri": tri,
            "bsc": bsc,
        }
        for i in range(NCORES)
    ]


def _run(input_data, w, b, trace=False):
    nc = _get_nc()
    in_maps = _prep_in_maps(input_data, w, b)
    res = run_bass_kernel_spmd(
        nc, in_maps, core_ids=list(range(NCORES)), trace=trace
    )
    out = np.concatenate([res.results[i]["out"] for i in range(NCORES)], axis=0)
    return out.astype(np.float32, copy=False), res


def kernel(input_data, w, b):
    out, _ = _run(input_data, w, b, trace=False)
    return out

